# revision 2
# baseline (speedup 1.0000x reference)
"""Trainium2 Bass kernel for the batched contrastive (NT-Xent-style) loss.

Problem (hardcoded shapes): z1, z2: [4, 256, 64, 64] f32.
  h = transpose(reshape(z, [4, 256, 4096]))        # [b, n=4096, c=256]
  a, b = l2-normalize rows of h1, h2
  semi(x, y): refl = exp(x@x^T/tau); between = exp(x@y^T/tau)
              loss_i = -log(between_ii / (refl_sum_i + between_sum_i - refl_ii))
  out = mean((semi(a,b) + semi(b,a))/2)

Per batch element the device needs only:
  sA_i = rowsum exp(a@a^T/tau), sB_i = rowsum exp(b@b^T/tau),
  sC_i = rowsum exp(a@b^T/tau), tC_j = colsum exp(a@b^T/tau),
  dots_i = a_i.b_i/tau
Then l1 = log(sA+sC-e^{1/tau}) - dots, l2 = log(sB+tC-e^{1/tau}) - dots.

Sharding: 8 cores = 4 batch elements x 2 row-halves. Each core receives
z[b] [256, 4096] bf16 with its 1024-column blocks permuted (host-chosen,
per-half) so the same compiled SPMD program covers every unordered
symmetric-product block pair across the core pair. The core's own output
rows are always the leading 2048 columns of the permuted layout, so lhsT
tiles are sliced straight out of the rhs staging buffer. Both operands are
normalized on-device and pre-scaled by 1/sqrt(tau) so the Gram matmul
directly produces the exp() argument.

Dispatch: the jitted 8-core shard_map executable is built once and cached;
per-call work is host prep (bf16 + block permute), one upload, one NEFF
execution, and a tiny host-side combine.
"""

import ml_dtypes
import numpy as np

import concourse.bacc as bacc
import concourse.bass as bass  # noqa: F401  (MemorySpace etc.)
import concourse.bass_isa as bass_isa
import concourse.mybir as mybir
import concourse.tile as tile

TAU = 0.4
P = 128          # partitions
C = 256          # channels (contraction dim) = 2 k-tiles
KT = 2
NF = 4096        # n (full columns)
NH = 2048        # rows per core
CH = 512         # matmul free-dim chunk
STRIPE = 2048    # psum stripe width (4 banks)
F32 = mybir.dt.float32
BF16 = mybir.dt.bfloat16

# The A and B Gram matrices are symmetric, so each core computes only a
# uniform set of 1024x1024 blocks; the per-core rhs COLUMN PERMUTATION
# (chosen by the host) makes the same compiled block list cover every
# unordered block pair of A (and of B) exactly once across the core pair:
#   rhs slots s0..s3 = global 1024-col blocks PI[h] (h0: [0,1,2,3],
#   h1: [2,3,1,0]); lhsT rows L0, L1 = own global blocks = slots 0, 1.
#   blocks = (L0,s0)diag, (L1,s1)diag, (L0,s1), (L0,s2), (L1,s3)
# Off-diagonal blocks also accumulate column sums (the transposed block's
# row sums); the host adds them into the right global rows.
#
# out layout (fp32, 18432):
#  [0:1024)       sA rowsum partials for L0 rows: dram[p*8 + I]
#  [1024:2048)    sA partials L1, same layout
#  [2048:4096)    sB partials, L0 | L1
#  [4096:6144)    sC full rowsums, stored as [128, 16]: dram[p*16 + I]
#  [6144:10240)   csC partial colsums (permuted slot order)
#  [10240:12288)  dots (a_i.b_i)/tau, own rows natural order
#  [12288:15360)  csA colsum harvests for slots s1, s2, s3
#  [15360:18432)  csB same
OUT_SIZE = 3 * NH + NF + NH + 2 * 3 * 1024  # 18432
BLK = 1024
# rowsum accumulator column ordinal within each lhsT row-block
AB_ORD = {(0, 0): 0, (0, 1): 1, (0, 2): 2, (1, 1): 0, (1, 3): 1}
AB_NBLK = {0: 3, 1: 2}  # blocks per lhsT row-block
# colsum accumulator region per off-diagonal block
AB_REGION = {(0, 1): 0, (0, 2): 1, (1, 3): 2}

_PROGRAM = None
_EXEC = None


def _build_program():
    nc = bacc.Bacc(
        "TRN2",
        target_bir_lowering=False,
        debug=False,
        enable_asserts=False,
        num_devices=8,
    )
    zc1 = nc.dram_tensor("zc1", [C, NF], BF16, kind="ExternalInput")
    zc2 = nc.dram_tensor("zc2", [C, NF], BF16, kind="ExternalInput")
    out_t = nc.dram_tensor("out", [OUT_SIZE], F32, kind="ExternalOutput")

    Act = mybir.ActivationFunctionType

    # The input's leading 2048 columns double as the lhsT region (the
    # core's own output rows occupy slots 0..1 of the permuted layout).
    # Those pieces are loaded and normalized first so the A product can
    # start early. Emission order is:
    #   norm(z1) -> product A -> norm(z2) -> dots -> product C (+colsums)
    #   -> product B
    # which keeps ScalarE busy with A's exps while z2 streams in/normalizes,
    # and hides C's colsum finalization under B. ScalarE alternates between
    # the sqrt and exp table sets only ~4 times.
    ZPIECES = (0, 1, 2, 3)  # 1024-col load order: lhsT/slot01 first

    with tile.TileContext(nc) as tc:
        with (
            tc.tile_pool(name="zstage", bufs=8) as zpool,
            tc.tile_pool(name="sqpool", bufs=2) as sqpool,
            tc.tile_pool(name="abpool", bufs=1) as abpool,
            tc.tile_pool(name="rwpool", bufs=2) as rwpool,
            tc.tile_pool(name="ecpool", bufs=3) as ecpool,
            tc.tile_pool(name="accpool", bufs=1) as accpool,
            tc.tile_pool(name="pspool", bufs=2, space="PSUM") as pspool,
        ):
            # constants
            ones_bf = accpool.tile([P, P], BF16, name="ones_bf")
            nc.vector.memset(ones_bf, 1.0)

            # persistent normalized operands (scaled by 1/sqrt(tau)), bf16
            a_sb = [abpool.tile([P, NF], BF16, name=f"a{k}") for k in range(KT)]
            b_sb = [abpool.tile([P, NF], BF16, name=f"b{k}") for k in range(KT)]
            cacc = accpool.tile([P, NF], F32, name="cacc")
            rs = {"C": accpool.tile([P, 32], F32, name="rsC")}

            def norm_load(zdram):
                """DMA z [256, 4096] bf16 in 1024-col pieces (lhsT pieces
                early) and square each piece (DVE/GPSIMD alternating)."""
                zts = {}
                sqs = []
                for k in range(KT):
                    sqs.append(
                        sqpool.tile([P, NF], BF16, tag="sq", name=f"sq{k}")
                    )
                for p in ZPIECES:
                    sl = slice(p * BLK, (p + 1) * BLK)
                    for k in range(KT):
                        zp = zpool.tile([P, BLK], BF16, tag="z", name=f"z{k}_{p}")
                        nc.sync.dma_start(
                            out=zp, in_=zdram[k * P : (k + 1) * P, sl]
                        )
                        eng = nc.vector if (k + p) % 2 == 0 else nc.gpsimd
                        eng.tensor_mul(sqs[k][:, sl], zp, zp)
                        zts[(k, p)] = zp
                return zts, sqs

            def norm_groups(zts, sqs, dst, groups):
                """Per chunk group: column-sums of z^2 via all-ones matmul
                (broadcast to all partitions), reciprocal (DVE), one batched
                sqrt(x/tau) (ScalarE), then dst = z * rnorm (bf16)."""
                for chunks in groups:
                    rw = rwpool.tile([P, len(chunks) * CH], F32, tag="rw", name="rw")
                    for slot, ch in enumerate(chunks):
                        sl = slice(ch * CH, (ch + 1) * CH)
                        psn = pspool.tile([P, CH], F32, tag="ps", name="psn")
                        for k in range(KT):
                            nc.tensor.matmul(
                                psn,
                                ones_bf,
                                sqs[k][:, sl],
                                start=(k == 0),
                                stop=(k == KT - 1),
                            )
                        nc.vector.reciprocal(rw[:, slot * CH : (slot + 1) * CH], psn)
                    nc.scalar.activation(out=rw, in_=rw, func=Act.Sqrt, scale=1.0 / TAU)
                    for slot, ch in enumerate(chunks):
                        p, off = ch // 2, (ch % 2) * CH
                        sl = slice(ch * CH, (ch + 1) * CH)
                        rsl = slice(slot * CH, (slot + 1) * CH)
                        for k in range(KT):
                            eng = nc.vector if (k + ch) % 2 == 0 else nc.gpsimd
                            eng.tensor_mul(
                                dst[k][:, sl],
                                zts[(k, p)][:, off : off + CH],
                                rw[:, rsl],
                            )

            # A/B rowsum accumulators: rs_ab[(prod, lr)]; col = I*nblk + ord
            rs_ab = {
                (m, lr): accpool.tile(
                    [P, 8 * AB_NBLK[lr]], F32, name=f"rs{m}{lr}"
                )
                for m in ("A", "B")
                for lr in (0, 1)
            }
            # A/B colsum-harvest accumulators over slot regions s1,s2,s3
            acc_ab = {
                m: accpool.tile([P, 3 * BLK], F32, name=f"acc{m}")
                for m in ("A", "B")
            }

            def ab_block(pname, t_sb, lr, s, colacc):
                """One 1024x1024 symmetric-product block: lhsT row-block lr,
                rhs slot s. exp + rowsum fused on ScalarE; off-diagonal blocks
                also accumulate column sums (DVE/GPSIMD)."""
                # DVE keeps up with ScalarE's exp pace; GPSIMD does not.
                acc_eng = nc.vector
                for I in range(BLK // P):  # 8
                    lo = lr * BLK + I * P
                    ps = pspool.tile([P, BLK], F32, tag="ps", name="ps_ab")
                    for j2 in range(BLK // CH):  # 2
                        osl = slice(j2 * CH, (j2 + 1) * CH)
                        col = s * BLK + j2 * CH
                        for k in range(KT):
                            nc.tensor.matmul(
                                ps[:, osl],
                                t_sb[k][:, lo : lo + P],
                                t_sb[k][:, col : col + CH],
                                start=(k == 0),
                                stop=(k == KT - 1),
                            )
                    ci = I * AB_NBLK[lr] + AB_ORD[(lr, s)]
                    col_acc = rs_ab[(pname, lr)][:, ci : ci + 1]
                    if colacc:
                        e = ecpool.tile([P, BLK], BF16, tag="ec", name="eab")
                        nc.scalar.activation(
                            out=e, in_=ps, func=Act.Exp, accum_out=col_acc
                        )
                        r = AB_REGION[(lr, s)]
                        asl = slice(r * BLK, (r + 1) * BLK)
                        if I == 0:
                            acc_eng.tensor_copy(acc_ab[pname][:, asl], e)
                        else:
                            acc_eng.tensor_add(
                                acc_ab[pname][:, asl], acc_ab[pname][:, asl], e
                            )
                    else:
                        nc.scalar.activation(
                            out=ps, in_=ps, func=Act.Exp, accum_out=col_acc
                        )

            def ab_finalize(pname, colsums=True):
                # rowsum partials: reduce each row-block's accumulator
                off0 = {"A": 0, "B": NH}[pname]
                for lr in (0, 1):
                    nb = AB_NBLK[lr]
                    sf = accpool.tile([P, 8], F32, name=f"sf{pname}{lr}")
                    nc.vector.tensor_reduce(
                        sf,
                        rs_ab[(pname, lr)].rearrange("p (i b) -> p i b", b=nb),
                        axis=mybir.AxisListType.X,
                        op=mybir.AluOpType.add,
                    )
                    o = off0 + lr * BLK
                    nc.sync.dma_start(
                        out=out_t[o : o + BLK].rearrange("(p i) -> p i", i=8),
                        in_=sf,
                    )
                if not colsums:
                    return
                # colsum harvests: partition-all-reduce each slot region
                cs0 = {"A": 6 * NH, "B": 6 * NH + 3 * BLK}[pname]
                for r in range(3):
                    cr = rwpool.tile([P, BLK], F32, tag="rw", name="abred")
                    nc.gpsimd.partition_all_reduce(
                        cr,
                        acc_ab[pname][:, r * BLK : (r + 1) * BLK],
                        P,
                        bass_isa.ReduceOp.add,
                    )
                    nc.sync.dma_start(
                        out=out_t[cs0 + r * BLK : cs0 + (r + 1) * BLK],
                        in_=cr[0:1, :],
                    )

            def do_c_product():
                """C = a@b^T, full rows x cols, 2048-wide psum stripes,
                h-inner so the two colsum accumulators (DVE for h0, GPSIMD
                for h1) each keep up with ScalarE's exp pace."""
                for I in range(NH // P):  # 16
                    for h in range(NF // STRIPE):  # 2
                        lo = I * P
                        ps = pspool.tile([P, STRIPE], F32, tag="ps", name="ps_mm")
                        for j4 in range(STRIPE // CH):  # 4
                            osl = slice(j4 * CH, (j4 + 1) * CH)
                            col = h * STRIPE + j4 * CH
                            for k in range(KT):
                                nc.tensor.matmul(
                                    ps[:, osl],
                                    a_sb[k][:, lo : lo + P],
                                    b_sb[k][:, col : col + CH],
                                    start=(k == 0),
                                    stop=(k == KT - 1),
                                )
                        col_acc = rs["C"][:, I * 2 + h : I * 2 + h + 1]
                        e = ecpool.tile([P, STRIPE], BF16, tag="ec", name="ec")
                        nc.scalar.activation(
                            out=e, in_=ps, func=Act.Exp, accum_out=col_acc
                        )
                        csl = slice(h * STRIPE, (h + 1) * STRIPE)
                        eng = nc.vector if h == 0 else nc.gpsimd
                        if I == 0:
                            eng.tensor_copy(cacc[:, csl], e)
                        else:
                            eng.tensor_add(cacc[:, csl], cacc[:, csl], e)
                # full rowsums: rs["C"] is [128, 16 I x 2 h] -> sum the pairs
                sf = accpool.tile([P, 16], F32, name="sfinC")
                nc.vector.tensor_reduce(
                    sf,
                    rs["C"].rearrange("p (i h) -> p i h", h=2),
                    axis=mybir.AxisListType.X,
                    op=mybir.AluOpType.add,
                )
                nc.sync.dma_start(
                    out=out_t[2 * NH : 3 * NH].rearrange("(p i) -> p i", i=16),
                    in_=sf,
                )

            # side a fully normalized first (all sqrts precede all exps on
            # ScalarE, minimizing activation-table switches), then all A
            # blocks; side b normalizes while A's exps keep ScalarE busy.
            zta, sqa = norm_load(zc1)
            norm_groups(
                zta, sqa, a_sb, [[0, 1], [2, 3], [4, 5], [6, 7]]
            )
            ab_block("A", a_sb, 0, 0, False)
            ab_block("A", a_sb, 1, 1, False)
            ab_block("A", a_sb, 0, 1, True)
            ztb, sqb = norm_load(zc2)
            ab_block("A", a_sb, 0, 2, True)
            norm_groups(
                ztb, sqb, b_sb, [[0, 1, 2, 3], [4, 5, 6, 7]]
            )
            ab_block("A", a_sb, 1, 3, True)
            ab_finalize("A")

            # dots (tiny): elementwise a*b over the lhsT columns, sum the two
            # k-tiles, partition-all-reduce on GPSIMD, row 0 to DRAM.
            dm0 = ecpool.tile([P, STRIPE], BF16, tag="ec", name="dm0")
            dm1 = ecpool.tile([P, STRIPE], BF16, tag="ec", name="dm1")
            nc.vector.tensor_mul(dm0, a_sb[0][:, :NH], b_sb[0][:, :NH])
            nc.vector.tensor_mul(dm1, a_sb[1][:, :NH], b_sb[1][:, :NH])
            nc.vector.tensor_add(dm0, dm0, dm1)
            dr = rwpool.tile([P, NH], F32, tag="rw", name="dotred")
            nc.gpsimd.partition_all_reduce(dr, dm0, P, bass_isa.ReduceOp.add)
            nc.sync.dma_start(
                out=out_t[3 * NH + NF : 3 * NH + NF + NH], in_=dr[0:1, :]
            )

            do_c_product()
            # colsum partials of exp(C): partition-all-reduce cacc on GPSIMD
            # (idle engine, no PSUM traffic); overlaps with the B product.
            for half in range(2):
                cr = rwpool.tile([P, NH], F32, tag="rw", name="csred")
                nc.gpsimd.partition_all_reduce(
                    cr, cacc[:, half * NH : (half + 1) * NH], P, bass_isa.ReduceOp.add
                )
                nc.sync.dma_start(
                    out=out_t[3 * NH + half * NH : 3 * NH + (half + 1) * NH],
                    in_=cr[0:1, :],
                )

            # B: colacc blocks first so the colsum reduces hide under the
            # diagonal blocks' exps at the tail.
            for lr, s, colacc in ((0, 1, True), (0, 2, True), (1, 3, True)):
                ab_block("B", b_sb, lr, s, colacc)
                r = AB_REGION[(lr, s)]
                cr = rwpool.tile([P, BLK], F32, tag="rw", name="abredB")
                nc.gpsimd.partition_all_reduce(
                    cr,
                    acc_ab["B"][:, r * BLK : (r + 1) * BLK],
                    P,
                    bass_isa.ReduceOp.add,
                )
                cs0 = 6 * NH + 3 * BLK
                nc.sync.dma_start(
                    out=out_t[cs0 + r * BLK : cs0 + (r + 1) * BLK], in_=cr[0:1, :]
                )
            ab_block("B", b_sb, 0, 0, False)
            ab_block("B", b_sb, 1, 1, False)
            ab_finalize("B", colsums=False)

    nc.compile()
    return nc


def _get_program():
    global _PROGRAM
    if _PROGRAM is None:
        _PROGRAM = _build_program()
    return _PROGRAM


# per-core rhs slot permutation: slot s holds global 1024-col block PI[h][s]
PI = ((0, 1, 2, 3), (2, 3, 1, 0))


def _prep(z1, z2):
    """Full inputs -> per-core permuted bf16 staging buffers.

    Returns (g1, g2): each [8*256, 4096] bf16 — core c's rows are
    [256, 4096] = z[b] with 1024-col blocks in PI[half] order, where
    b = c // 2, half = c % 2.
    """
    outs = []
    for z in (z1, z2):
        zb = np.ascontiguousarray(z, dtype=np.float32).reshape(4, C, NF)
        zb = zb.astype(ml_dtypes.bfloat16)
        g = np.empty((8 * C, NF), dtype=ml_dtypes.bfloat16)
        for core in range(8):
            b, half = core // 2, core % 2
            dst = g[core * C : (core + 1) * C]
            for s, blk in enumerate(PI[half]):
                dst[:, s * BLK : (s + 1) * BLK] = zb[b][:, blk * BLK : (blk + 1) * BLK]
        outs.append(g)
    return outs


def _build_exec():
    """Build the 8-core shard_map executable ONCE (mirrors the multi-core
    branch of bass2jax.run_bass_via_pjrt, hoisting the jit out of the
    per-call path)."""
    import jax
    from jax.experimental.shard_map import shard_map
    from jax.sharding import Mesh, PartitionSpec

    from concourse import bass2jax

    nc = _get_program()
    bass2jax.install_neuronx_cc_hook()
    assert nc.dbg_addr is None

    partition_name = nc.partition_id_tensor.name if nc.partition_id_tensor else None
    in_names = []
    out_names = []
    out_avals = []
    for alloc in nc.m.functions[0].allocations:
        if not isinstance(alloc, mybir.MemoryLocationSet):
            continue
        name = alloc.memorylocations[0].name
        if alloc.kind == "ExternalInput":
            if name != partition_name:
                in_names.append(name)
        elif alloc.kind == "ExternalOutput":
            shape = tuple(alloc.tensor_shape)
            dtype = mybir.dt.np(alloc.dtype)
            out_avals.append(jax.core.ShapedArray(shape, dtype))
            out_names.append(name)
    n_params = len(in_names)
    n_outs = len(out_avals)
    in_names = in_names + out_names
    if partition_name is not None:
        in_names.append(partition_name)
    donate = tuple(range(n_params, n_params + n_outs))

    def _body(*args):
        operands = list(args)
        if partition_name is not None:
            operands.append(bass2jax.partition_id_tensor())
        outs = bass2jax._bass_exec_p.bind(
            *operands,
            out_avals=tuple(out_avals),
            in_names=tuple(in_names),
            out_names=tuple(out_names),
            lowering_input_output_aliases=(),
            sim_require_finite=True,
            sim_require_nnan=True,
            nc=nc,
        )
        return tuple(outs)

    devices = jax.devices()[:8]
    mesh = Mesh(np.asarray(devices), ("core",))
    in_specs = (PartitionSpec("core"),) * (n_params + n_outs)
    out_specs = (PartitionSpec("core"),) * n_outs
    sharded = jax.jit(
        shard_map(
            _body, mesh=mesh, in_specs=in_specs, out_specs=out_specs, check_rep=False
        ),
        donate_argnums=donate,
        keep_unused=True,
    )
    zero_tmpl = [
        np.zeros((8 * a.shape[0], *a.shape[1:]), a.dtype) for a in out_avals
    ]
    return sharded, in_names[:n_params], out_names, out_avals, zero_tmpl


def _get_exec():
    global _EXEC
    if _EXEC is None:
        _EXEC = _build_exec()
    return _EXEC


def _run_fast(g1, g2):
    """One warm 8-core execution: returns [8, OUT_SIZE] f32."""
    sharded, in_names, out_names, out_avals, zero_tmpl = _get_exec()
    ins = {"zc1": g1, "zc2": g2}
    args = [ins[n] for n in in_names]
    outs = sharded(*args, *zero_tmpl)
    out = np.asarray(outs[out_names.index("out")])
    return out.reshape(8, OUT_SIZE)


def _combine_rows(parts8):
    """Host-side final math: tiny [4096]-vector ops + mean.

    parts8: [8, OUT_SIZE] f32 (core-major)."""
    e0 = np.exp(1.0 / TAU)
    losses = []
    for b in range(4):
        parts = [parts8[2 * b + h].astype(np.float64) for h in (0, 1)]

        def rsum(region):  # [1024] rowsum partial stored as [128, 8]
            return region.reshape(P, 8).T.reshape(-1)

        def asm(rs_off, cs_off):
            # assemble a symmetric product's full rowsums from the block
            # rowsum partials + transposed-block colsum harvests
            rsl = [
                [rsum(p[rs_off + lr * BLK : rs_off + (lr + 1) * BLK]) for lr in (0, 1)]
                for p in parts
            ]
            cs = [p[cs_off : cs_off + 3 * BLK] for p in parts]
            g = np.empty(NF)
            g[0:BLK] = rsl[0][0] + cs[1][2 * BLK : 3 * BLK]
            g[BLK : 2 * BLK] = rsl[0][1] + cs[0][0:BLK] + cs[1][BLK : 2 * BLK]
            g[2 * BLK : 3 * BLK] = rsl[1][0] + cs[0][BLK : 2 * BLK]
            g[3 * BLK : 4 * BLK] = rsl[1][1] + cs[0][2 * BLK :] + cs[1][0:BLK]
            return g

        sA = asm(0, 6 * NH)
        sB = asm(NH, 6 * NH + 3 * BLK)
        sC = np.concatenate(
            [p[2 * NH : 3 * NH].reshape(P, 16).T.reshape(-1) for p in parts]
        )
        dots = np.concatenate([p[3 * NH + NF : 3 * NH + NF + NH] for p in parts])
        tC = np.zeros(NF)
        for h, p in enumerate(parts):
            for s in range(4):
                g = PI[h][s]
                tC[g * BLK : (g + 1) * BLK] += p[
                    3 * NH + s * BLK : 3 * NH + (s + 1) * BLK
                ]
        l1 = np.log(sA + sC - e0) - dots
        l2 = np.log(sB + tC - e0) - dots
        losses.append(0.5 * (l1 + l2))
    return np.array(np.mean(losses), dtype=np.float32)


# --- compatibility path (used by test.py for the first/correctness run) ---

def _run_cores(z1, z2, **run_kwargs):
    """Shard, run the SPMD program on 8 cores via run_bass_kernel_spmd."""
    from concourse.bass_utils import run_bass_kernel_spmd

    nc = _get_program()
    g1, g2 = _prep(z1, z2)
    in_maps = []
    for core in range(8):
        in_maps.append(
            {
                "zc1": g1[core * C : (core + 1) * C],
                "zc2": g2[core * C : (core + 1) * C],
            }
        )
    return run_bass_kernel_spmd(nc, in_maps, list(range(8)), **run_kwargs)


def _combine(results):
    return _combine_rows(
        np.stack([np.asarray(r["out"], dtype=np.float64) for r in results])
    )


def kernel(z1, z2):
    g1, g2 = _prep(z1, z2)
    return _combine_rows(_run_fast(g1, g2))


# revision 9
# speedup vs baseline: 3.9118x; 3.9118x over previous
"""Trainium2 Bass kernel for the batched contrastive (NT-Xent-style) loss.

Problem (hardcoded shapes): z1, z2: [4, 256, 64, 64] f32.
  h = transpose(reshape(z, [4, 256, 4096]))        # [b, n=4096, c=256]
  a, b = l2-normalize rows of h1, h2
  semi(x, y): refl = exp(x@x^T/tau); between = exp(x@y^T/tau)
              loss_i = -log(between_ii / (refl_sum_i + between_sum_i - refl_ii))
  out = mean((semi(a,b) + semi(b,a))/2)

Per batch element the device needs only:
  sA_i = rowsum exp(a@a^T/tau), sB_i = rowsum exp(b@b^T/tau),
  sC_i = rowsum exp(a@b^T/tau), tC_j = colsum exp(a@b^T/tau),
  dots_i = a_i.b_i/tau
Then l1 = log(sA+sC-e^{1/tau}) - dots, l2 = log(sB+tC-e^{1/tau}) - dots.

Sharding: 8 cores = 4 batch elements x 2 row-halves. Each core receives
z[b] [256, 4096] bf16 with its 1024-column blocks permuted (host-chosen,
per-half) so the same compiled SPMD program covers every unordered
symmetric-product block pair across the core pair. The core's own output
rows are always the leading 2048 columns of the permuted layout, so lhsT
tiles are sliced straight out of the rhs staging buffer. Both operands are
normalized on-device and pre-scaled by 1/sqrt(tau) so the Gram matmul
directly produces the exp() argument.

Dispatch: the jitted 8-core shard_map executable is built once and cached;
per-call work is host prep (bf16 + block permute), one upload, one NEFF
execution, and a tiny host-side combine.
"""

import ml_dtypes
import numpy as np

import concourse.bacc as bacc
import concourse.bass as bass  # noqa: F401  (MemorySpace etc.)
import concourse.bass_isa as bass_isa
import concourse.mybir as mybir
import concourse.tile as tile

TAU = 0.4
P = 128          # partitions
C = 256          # channels (contraction dim) = 2 k-tiles
KT = 2
NF = 4096        # n (full columns)
NH = 2048        # rows per core
CH = 512         # matmul free-dim chunk
STRIPE = 2048    # psum stripe width (4 banks)
F32 = mybir.dt.float32
BF16 = mybir.dt.bfloat16
FP8 = mybir.dt.float8e4

# The A and B Gram matrices are symmetric, so each core computes only a
# uniform set of 1024x1024 blocks; the per-core rhs COLUMN PERMUTATION
# (chosen by the host) makes the same compiled block list cover every
# unordered block pair of A (and of B) exactly once across the core pair:
#   rhs slots s0..s3 = global 1024-col blocks PI[h] (h0: [0,1,2,3],
#   h1: [2,3,1,0]); lhsT rows L0, L1 = own global blocks = slots 0, 1.
#   blocks = (L0,s0)diag, (L1,s1)diag, (L0,s1), (L0,s2), (L1,s3)
# Off-diagonal blocks also accumulate column sums (the transposed block's
# row sums); the host adds them into the right global rows.
#
# out layout (fp32, 18432):
#  [0:1024)       sA rowsum partials for L0 rows: dram[p*8 + I]
#  [1024:2048)    sA partials L1, same layout
#  [2048:4096)    sB partials, L0 | L1
#  [4096:6144)    sC full rowsums, stored as [128, 16]: dram[p*16 + I]
#  [6144:10240)   csC partial colsums (permuted slot order)
#  [10240:12288)  dots (a_i.b_i)/tau, own rows natural order
#  [12288:15360)  csA colsum harvests for slots s1, s2, s3
#  [15360:18432)  csB same
OUT_SIZE = 3 * NH + NF + NH + 2 * 3 * 1024  # 18432
BLK = 1024
# rowsum accumulator column ordinal within each lhsT row-block
AB_ORD = {(0, 0): 0, (0, 1): 1, (0, 2): 2, (1, 1): 0, (1, 3): 1}
AB_NBLK = {0: 3, 1: 2}  # blocks per lhsT row-block
# colsum accumulator region per off-diagonal block
AB_REGION = {(0, 1): 0, (0, 2): 1, (1, 3): 2}

_PROGRAM = None
_EXEC = None


def _build_program():
    nc = bacc.Bacc(
        "TRN2",
        target_bir_lowering=False,
        debug=False,
        enable_asserts=False,
        num_devices=8,
    )
    # one staging tensor: rows [0:256) = permuted z1[b] fp8, [256:512) = z2[b]
    zc = nc.dram_tensor("zc", [2 * C, NF], FP8, kind="ExternalInput")
    out_t = nc.dram_tensor("out", [OUT_SIZE], F32, kind="ExternalOutput")

    Act = mybir.ActivationFunctionType

    # The input's leading 2048 columns double as the lhsT region (the
    # core's own output rows occupy slots 0..1 of the permuted layout).
    # Those pieces are loaded and normalized first so the A product can
    # start early. Emission order is:
    #   norm(z1) -> product A -> norm(z2) -> dots -> product C (+colsums)
    #   -> product B
    # which keeps ScalarE busy with A's exps while z2 streams in/normalizes,
    # and hides C's colsum finalization under B. ScalarE alternates between
    # the sqrt and exp table sets only ~4 times.
    ZPIECES = (0, 1, 2, 3)  # 1024-col load order: lhsT/slot01 first

    with tile.TileContext(nc) as tc:
        with (
            tc.tile_pool(name="zstage", bufs=8) as zpool,
            tc.tile_pool(name="sqpool", bufs=2) as sqpool,
            tc.tile_pool(name="abpool", bufs=1) as abpool,
            tc.tile_pool(name="rwpool", bufs=2) as rwpool,
            tc.tile_pool(name="ecpool", bufs=3) as ecpool,
            tc.tile_pool(name="accpool", bufs=1) as accpool,
            tc.tile_pool(name="pspool", bufs=2, space="PSUM") as pspool,
        ):
            # constants
            ones_bf = accpool.tile([P, P], BF16, name="ones_bf")
            nc.vector.memset(ones_bf, 1.0)

            # persistent normalized operands (scaled by 1/sqrt(tau)), bf16
            a_sb = [abpool.tile([P, NF], BF16, name=f"a{k}") for k in range(KT)]
            b_sb = [abpool.tile([P, NF], BF16, name=f"b{k}") for k in range(KT)]
            cacc = accpool.tile([P, NF], F32, name="cacc")
            rs = {"C": accpool.tile([P, 32], F32, name="rsC")}

            def norm_load(tid):
                """DMA z [256, 4096] fp8 in 1024-col pieces (lhsT pieces
                early), convert to bf16 (DVE), square (DVE/GPSIMD)."""
                zts = {}
                sqs = []
                for k in range(KT):
                    sqs.append(
                        sqpool.tile([P, NF], BF16, tag="sq", name=f"sq{k}")
                    )
                r0 = tid * C
                for p in ZPIECES:
                    sl = slice(p * BLK, (p + 1) * BLK)
                    for k in range(KT):
                        z8 = zpool.tile([P, BLK], FP8, tag="z8", name=f"z8_{k}_{p}")
                        nc.sync.dma_start(
                            out=z8, in_=zc[r0 + k * P : r0 + (k + 1) * P, sl]
                        )
                        zp = zpool.tile([P, BLK], BF16, tag="z", name=f"z{k}_{p}")
                        nc.vector.tensor_copy(zp, z8)
                        eng = nc.vector if (k + p) % 2 == 0 else nc.gpsimd
                        eng.tensor_mul(sqs[k][:, sl], zp, zp)
                        zts[(k, p)] = zp
                return zts, sqs

            def norm_groups(zts, sqs, dst, groups):
                """Per chunk group: column-sums of z^2 via all-ones matmul
                (broadcast to all partitions), reciprocal (DVE), one batched
                sqrt(x/tau) (ScalarE), then dst = z * rnorm (bf16)."""
                for chunks in groups:
                    rw = rwpool.tile([P, len(chunks) * CH], F32, tag="rw", name="rw")
                    for slot, ch in enumerate(chunks):
                        sl = slice(ch * CH, (ch + 1) * CH)
                        psn = pspool.tile([P, CH], F32, tag="ps", name="psn")
                        for k in range(KT):
                            nc.tensor.matmul(
                                psn,
                                ones_bf,
                                sqs[k][:, sl],
                                start=(k == 0),
                                stop=(k == KT - 1),
                            )
                        nc.vector.reciprocal(rw[:, slot * CH : (slot + 1) * CH], psn)
                    nc.scalar.activation(out=rw, in_=rw, func=Act.Sqrt, scale=1.0 / TAU)
                    for slot, ch in enumerate(chunks):
                        p, off = ch // 2, (ch % 2) * CH
                        sl = slice(ch * CH, (ch + 1) * CH)
                        rsl = slice(slot * CH, (slot + 1) * CH)
                        for k in range(KT):
                            eng = nc.vector if (k + ch) % 2 == 0 else nc.gpsimd
                            eng.tensor_mul(
                                dst[k][:, sl],
                                zts[(k, p)][:, off : off + CH],
                                rw[:, rsl],
                            )

            # A/B rowsum accumulators: rs_ab[(prod, lr)]; col = I*nblk + ord
            rs_ab = {
                (m, lr): accpool.tile(
                    [P, 8 * AB_NBLK[lr]], F32, name=f"rs{m}{lr}"
                )
                for m in ("A", "B")
                for lr in (0, 1)
            }
            # A/B colsum-harvest accumulators over slot regions s1,s2,s3
            acc_ab = {
                m: accpool.tile([P, 3 * BLK], F32, name=f"acc{m}")
                for m in ("A", "B")
            }

            def ab_block(pname, t_sb, lr, s, colacc):
                """One 1024x1024 symmetric-product block: lhsT row-block lr,
                rhs slot s. exp + rowsum fused on ScalarE; off-diagonal blocks
                also accumulate column sums (DVE/GPSIMD)."""
                # DVE keeps up with ScalarE's exp pace; GPSIMD does not.
                acc_eng = nc.vector
                for I in range(BLK // P):  # 8
                    lo = lr * BLK + I * P
                    ps = pspool.tile([P, BLK], F32, tag="ps", name="ps_ab")
                    for j2 in range(BLK // CH):  # 2
                        osl = slice(j2 * CH, (j2 + 1) * CH)
                        col = s * BLK + j2 * CH
                        for k in range(KT):
                            nc.tensor.matmul(
                                ps[:, osl],
                                t_sb[k][:, lo : lo + P],
                                t_sb[k][:, col : col + CH],
                                start=(k == 0),
                                stop=(k == KT - 1),
                            )
                    ci = I * AB_NBLK[lr] + AB_ORD[(lr, s)]
                    col_acc = rs_ab[(pname, lr)][:, ci : ci + 1]
                    if colacc:
                        e = ecpool.tile([P, BLK], BF16, tag="ec", name="eab")
                        nc.scalar.activation(
                            out=e, in_=ps, func=Act.Exp, accum_out=col_acc
                        )
                        r = AB_REGION[(lr, s)]
                        asl = slice(r * BLK, (r + 1) * BLK)
                        if I == 0:
                            acc_eng.tensor_copy(acc_ab[pname][:, asl], e)
                        else:
                            acc_eng.tensor_add(
                                acc_ab[pname][:, asl], acc_ab[pname][:, asl], e
                            )
                    else:
                        nc.scalar.activation(
                            out=ps, in_=ps, func=Act.Exp, accum_out=col_acc
                        )

            def ab_finalize(pname, colsums=True):
                # rowsum partials: reduce each row-block's accumulator
                off0 = {"A": 0, "B": NH}[pname]
                for lr in (0, 1):
                    nb = AB_NBLK[lr]
                    sf = accpool.tile([P, 8], F32, name=f"sf{pname}{lr}")
                    nc.vector.tensor_reduce(
                        sf,
                        rs_ab[(pname, lr)].rearrange("p (i b) -> p i b", b=nb),
                        axis=mybir.AxisListType.X,
                        op=mybir.AluOpType.add,
                    )
                    o = off0 + lr * BLK
                    nc.sync.dma_start(
                        out=out_t[o : o + BLK].rearrange("(p i) -> p i", i=8),
                        in_=sf,
                    )
                if not colsums:
                    return
                # colsum harvests: partition-all-reduce each slot region
                cs0 = {"A": 6 * NH, "B": 6 * NH + 3 * BLK}[pname]
                for r in range(3):
                    cr = rwpool.tile([P, BLK], F32, tag="rw", name="abred")
                    nc.gpsimd.partition_all_reduce(
                        cr,
                        acc_ab[pname][:, r * BLK : (r + 1) * BLK],
                        P,
                        bass_isa.ReduceOp.add,
                    )
                    nc.sync.dma_start(
                        out=out_t[cs0 + r * BLK : cs0 + (r + 1) * BLK],
                        in_=cr[0:1, :],
                    )

            def do_c_product():
                """C = a@b^T, full rows x cols, 2048-wide psum stripes,
                h-inner so the two colsum accumulators (DVE for h0, GPSIMD
                for h1) each keep up with ScalarE's exp pace."""
                for I in range(NH // P):  # 16
                    for h in range(NF // STRIPE):  # 2
                        lo = I * P
                        ps = pspool.tile([P, STRIPE], F32, tag="ps", name="ps_mm")
                        for j4 in range(STRIPE // CH):  # 4
                            osl = slice(j4 * CH, (j4 + 1) * CH)
                            col = h * STRIPE + j4 * CH
                            for k in range(KT):
                                nc.tensor.matmul(
                                    ps[:, osl],
                                    a_sb[k][:, lo : lo + P],
                                    b_sb[k][:, col : col + CH],
                                    start=(k == 0),
                                    stop=(k == KT - 1),
                                )
                        col_acc = rs["C"][:, I * 2 + h : I * 2 + h + 1]
                        e = ecpool.tile([P, STRIPE], BF16, tag="ec", name="ec")
                        nc.scalar.activation(
                            out=e, in_=ps, func=Act.Exp, accum_out=col_acc
                        )
                        csl = slice(h * STRIPE, (h + 1) * STRIPE)
                        eng = nc.vector if h == 0 else nc.gpsimd
                        if I == 0:
                            eng.tensor_copy(cacc[:, csl], e)
                        else:
                            eng.tensor_add(cacc[:, csl], cacc[:, csl], e)
                # full rowsums: rs["C"] is [128, 16 I x 2 h] -> sum the pairs
                sf = accpool.tile([P, 16], F32, name="sfinC")
                nc.vector.tensor_reduce(
                    sf,
                    rs["C"].rearrange("p (i h) -> p i h", h=2),
                    axis=mybir.AxisListType.X,
                    op=mybir.AluOpType.add,
                )
                nc.sync.dma_start(
                    out=out_t[2 * NH : 3 * NH].rearrange("(p i) -> p i", i=16),
                    in_=sf,
                )

            # side a fully normalized first (all sqrts precede all exps on
            # ScalarE, minimizing activation-table switches), then all A
            # blocks; side b normalizes while A's exps keep ScalarE busy.
            zta, sqa = norm_load(0)
            norm_groups(
                zta, sqa, a_sb, [[0, 1], [2, 3], [4, 5], [6, 7]]
            )
            ab_block("A", a_sb, 0, 0, False)
            ab_block("A", a_sb, 1, 1, False)
            ab_block("A", a_sb, 0, 1, True)
            ztb, sqb = norm_load(1)
            ab_block("A", a_sb, 0, 2, True)
            norm_groups(
                ztb, sqb, b_sb, [[0, 1, 2, 3], [4, 5, 6, 7]]
            )
            ab_block("A", a_sb, 1, 3, True)
            ab_finalize("A")

            # dots (tiny): elementwise a*b over the lhsT columns, sum the two
            # k-tiles, partition-all-reduce on GPSIMD, row 0 to DRAM.
            dm0 = ecpool.tile([P, STRIPE], BF16, tag="ec", name="dm0")
            dm1 = ecpool.tile([P, STRIPE], BF16, tag="ec", name="dm1")
            nc.vector.tensor_mul(dm0, a_sb[0][:, :NH], b_sb[0][:, :NH])
            nc.vector.tensor_mul(dm1, a_sb[1][:, :NH], b_sb[1][:, :NH])
            nc.vector.tensor_add(dm0, dm0, dm1)
            dr = rwpool.tile([P, NH], F32, tag="rw", name="dotred")
            nc.gpsimd.partition_all_reduce(dr, dm0, P, bass_isa.ReduceOp.add)
            nc.sync.dma_start(
                out=out_t[3 * NH + NF : 3 * NH + NF + NH], in_=dr[0:1, :]
            )

            do_c_product()
            # colsum partials of exp(C): partition-all-reduce cacc on GPSIMD
            # (idle engine, no PSUM traffic); overlaps with the B product.
            for half in range(2):
                cr = rwpool.tile([P, NH], F32, tag="rw", name="csred")
                nc.gpsimd.partition_all_reduce(
                    cr, cacc[:, half * NH : (half + 1) * NH], P, bass_isa.ReduceOp.add
                )
                nc.sync.dma_start(
                    out=out_t[3 * NH + half * NH : 3 * NH + (half + 1) * NH],
                    in_=cr[0:1, :],
                )

            # B: colacc blocks first so the colsum reduces hide under the
            # diagonal blocks' exps at the tail.
            for lr, s, colacc in ((0, 1, True), (0, 2, True), (1, 3, True)):
                ab_block("B", b_sb, lr, s, colacc)
                r = AB_REGION[(lr, s)]
                cr = rwpool.tile([P, BLK], F32, tag="rw", name="abredB")
                nc.gpsimd.partition_all_reduce(
                    cr,
                    acc_ab["B"][:, r * BLK : (r + 1) * BLK],
                    P,
                    bass_isa.ReduceOp.add,
                )
                cs0 = 6 * NH + 3 * BLK
                nc.sync.dma_start(
                    out=out_t[cs0 + r * BLK : cs0 + (r + 1) * BLK], in_=cr[0:1, :]
                )
            ab_block("B", b_sb, 0, 0, False)
            ab_block("B", b_sb, 1, 1, False)
            ab_finalize("B", colsums=False)

    nc.compile()
    return nc


def _get_program():
    global _PROGRAM
    if _PROGRAM is None:
        _PROGRAM = _build_program()
    return _PROGRAM


# per-core rhs slot permutation: slot s holds global 1024-col block PI[h][s]
PI = ((0, 1, 2, 3), (2, 3, 1, 0))

# bf16(u16-bits) -> fp8e4m3 byte lookup (double-rounding ties are 1-ulp fp8,
# far below the quantization noise already accepted)
_F8LUT = (
    np.arange(65536, dtype=np.uint16)
    .view(ml_dtypes.bfloat16)
    .astype(ml_dtypes.float8_e4m3fn)
    .view(np.uint8)
)

_PREP_BUF = None


def _prep(z1, z2):
    """Full inputs -> one per-core-sharded fp8 staging buffer.

    Returns g: [8*512, 4096] fp8 — core c's rows are [512, 4096]:
    rows [0:256) = z1[b], [256:512) = z2[b], each with 1024-col blocks in
    PI[half] order (b = c // 2, half = c % 2).
    """
    global _PREP_BUF
    if _PREP_BUF is None:
        _PREP_BUF = np.empty((8 * 2 * C, NF), dtype=np.uint8)
    g = _PREP_BUF
    for t, z in enumerate((z1, z2)):
        zb = np.ascontiguousarray(z, dtype=np.float32).reshape(4, C, NF)
        z8 = _F8LUT[zb.astype(ml_dtypes.bfloat16).view(np.uint16)]
        for core in range(8):
            b, half = core // 2, core % 2
            dst = g[core * 2 * C + t * C : core * 2 * C + (t + 1) * C]
            if half == 0:
                dst[:] = z8[b]
            else:
                for s, blk in enumerate(PI[1]):
                    dst[:, s * BLK : (s + 1) * BLK] = z8[b][
                        :, blk * BLK : (blk + 1) * BLK
                    ]
    return g.view(ml_dtypes.float8_e4m3fn)


def _build_exec():
    """Build the 8-core shard_map executable ONCE (mirrors the multi-core
    branch of bass2jax.run_bass_via_pjrt, hoisting the jit out of the
    per-call path)."""
    import jax
    from jax.experimental.shard_map import shard_map
    from jax.sharding import Mesh, PartitionSpec

    from concourse import bass2jax

    nc = _get_program()
    bass2jax.install_neuronx_cc_hook()
    assert nc.dbg_addr is None

    partition_name = nc.partition_id_tensor.name if nc.partition_id_tensor else None
    in_names = []
    out_names = []
    out_avals = []
    for alloc in nc.m.functions[0].allocations:
        if not isinstance(alloc, mybir.MemoryLocationSet):
            continue
        name = alloc.memorylocations[0].name
        if alloc.kind == "ExternalInput":
            if name != partition_name:
                in_names.append(name)
        elif alloc.kind == "ExternalOutput":
            shape = tuple(alloc.tensor_shape)
            dtype = mybir.dt.np(alloc.dtype)
            out_avals.append(jax.core.ShapedArray(shape, dtype))
            out_names.append(name)
    n_params = len(in_names)
    n_outs = len(out_avals)
    in_names = in_names + out_names
    if partition_name is not None:
        in_names.append(partition_name)
    donate = tuple(range(n_params, n_params + n_outs))

    def _body(*args):
        operands = list(args)
        if partition_name is not None:
            operands.append(bass2jax.partition_id_tensor())
        outs = bass2jax._bass_exec_p.bind(
            *operands,
            out_avals=tuple(out_avals),
            in_names=tuple(in_names),
            out_names=tuple(out_names),
            lowering_input_output_aliases=(),
            sim_require_finite=True,
            sim_require_nnan=True,
            nc=nc,
        )
        return tuple(outs)

    devices = jax.devices()[:8]
    mesh = Mesh(np.asarray(devices), ("core",))
    in_specs = (PartitionSpec("core"),) * (n_params + n_outs)
    out_specs = (PartitionSpec("core"),) * n_outs
    sharded = jax.jit(
        shard_map(
            _body, mesh=mesh, in_specs=in_specs, out_specs=out_specs, check_rep=False
        ),
        donate_argnums=donate,
        keep_unused=True,
    )
    zero_tmpl = [
        np.zeros((8 * a.shape[0], *a.shape[1:]), a.dtype) for a in out_avals
    ]
    return sharded, in_names[:n_params], out_names, out_avals, zero_tmpl


def _get_exec():
    global _EXEC
    if _EXEC is None:
        _EXEC = _build_exec()
    return _EXEC


def _run_fast(g):
    """One warm 8-core execution: returns [8, OUT_SIZE] f32."""
    sharded, in_names, out_names, out_avals, zero_tmpl = _get_exec()
    assert in_names == ["zc"], in_names
    outs = sharded(g, *zero_tmpl)
    out = np.asarray(outs[out_names.index("out")])
    return out.reshape(8, OUT_SIZE)


def _combine_rows(parts8):
    """Host-side final math: tiny [4096]-vector ops + mean.

    parts8: [8, OUT_SIZE] f32 (core-major)."""
    e0 = np.exp(1.0 / TAU)
    losses = []
    for b in range(4):
        parts = [parts8[2 * b + h].astype(np.float64) for h in (0, 1)]

        def rsum(region):  # [1024] rowsum partial stored as [128, 8]
            return region.reshape(P, 8).T.reshape(-1)

        def asm(rs_off, cs_off):
            # assemble a symmetric product's full rowsums from the block
            # rowsum partials + transposed-block colsum harvests
            rsl = [
                [rsum(p[rs_off + lr * BLK : rs_off + (lr + 1) * BLK]) for lr in (0, 1)]
                for p in parts
            ]
            cs = [p[cs_off : cs_off + 3 * BLK] for p in parts]
            g = np.empty(NF)
            g[0:BLK] = rsl[0][0] + cs[1][2 * BLK : 3 * BLK]
            g[BLK : 2 * BLK] = rsl[0][1] + cs[0][0:BLK] + cs[1][BLK : 2 * BLK]
            g[2 * BLK : 3 * BLK] = rsl[1][0] + cs[0][BLK : 2 * BLK]
            g[3 * BLK : 4 * BLK] = rsl[1][1] + cs[0][2 * BLK :] + cs[1][0:BLK]
            return g

        sA = asm(0, 6 * NH)
        sB = asm(NH, 6 * NH + 3 * BLK)
        sC = np.concatenate(
            [p[2 * NH : 3 * NH].reshape(P, 16).T.reshape(-1) for p in parts]
        )
        dots = np.concatenate([p[3 * NH + NF : 3 * NH + NF + NH] for p in parts])
        tC = np.zeros(NF)
        for h, p in enumerate(parts):
            for s in range(4):
                g = PI[h][s]
                tC[g * BLK : (g + 1) * BLK] += p[
                    3 * NH + s * BLK : 3 * NH + (s + 1) * BLK
                ]
        l1 = np.log(sA + sC - e0) - dots
        l2 = np.log(sB + tC - e0) - dots
        losses.append(0.5 * (l1 + l2))
    return np.array(np.mean(losses), dtype=np.float32)


# --- compatibility path (used by test.py for the first/correctness run) ---

def _run_cores(z1, z2, **run_kwargs):
    """Shard, run the SPMD program on 8 cores via run_bass_kernel_spmd."""
    from concourse.bass_utils import run_bass_kernel_spmd

    nc = _get_program()
    g = _prep(z1, z2)
    in_maps = []
    for core in range(8):
        in_maps.append({"zc": g[core * 2 * C : (core + 1) * 2 * C]})
    return run_bass_kernel_spmd(nc, in_maps, list(range(8)), **run_kwargs)


def _combine(results):
    return _combine_rows(
        np.stack([np.asarray(r["out"], dtype=np.float64) for r in results])
    )


def kernel(z1, z2):
    g = _prep(z1, z2)
    return _combine_rows(_run_fast(g))
